# revision 31
# baseline (speedup 1.0000x reference)
"""Trainium2 Bass kernel: bipartite GNN message passing (BranchingGNN), 8-core SPMD.

Sharding: core k owns constraint rows [k*6250,(k+1)*6250) and variable rows
[k*12500,(k+1)*12500); each core processes all edges targeting its shard, so
messages need no cross-core reduction. Node tables are compact row-major
[N, 64] bf16 in DRAM and are re-broadcast each phase by an AllGather of the
updated shard.

Per phase (one message direction):
  - edges sorted by (src-window, src-parity, dst-block, src-row); each
    (window,parity,block) group is padded to 128-edge tiles. Source rows are
    gathered by dma_gather (128B rows at 256B stride over the even/odd row
    subsequence of the compact table), ascending addresses for HBM locality.
  - per tile, a one-hot S [128e,128d] = (iota == dstcol) is built on DVE
    (batched per gather call); one PE matmul per tile accumulates
    msgT [64,128] into a per-group PSUM tile (start/stop over the group).
  - group partials are added into an SBUF accumulator macc per dst block;
    the block update relu(hT + W.T @ msgT + b) runs as soon as its last
    group lands; PE transpose + one DMA + AllGather republish the table.
"""
import sys

sys.path.insert(0, "/opt/trn_rl_repo")

import numpy as np
import ml_dtypes

import concourse.bass as bass
import concourse.bacc as bacc
import concourse.mybir as mybir
import concourse.tile as tile
from concourse.bass_utils import run_bass_kernel_spmd

# ---- problem constants
V, C, E = 100000, 50000, 1250000
VF, CF, H = 32, 32, 64
ROUNDS = 3
CORES = 8
P = 128
TPC = 7               # tiles per gather call (ring cap 64 descs)

V_CORE, C_CORE = 12500, 6250          # real nodes per core
V_S, C_S = 12544, 6272                # shard rows (98 / 49 blocks)
NB_V, NB_C = 98, 49                   # dst blocks per core
RV, RC = CORES * V_S, CORES * C_S     # 100352 / 50176 table rows
VWIN, CWIN = 2, 1                     # source windows (half-row reach 32767)
# chunk-major table layout: block ranges per table tensor (= gather window,
# = one AllGather each); plus finer DMA staging chunks
VCH = [0, 49, 98]
CCH = [0, 49]
VDM = [0, 25, 49, 74, 98]
CDM = [0, 13, 25, 37, 49]


def _row_map(n_core, chb):
    """node id -> chunk-major table row."""
    n = n_core * CORES
    v = np.arange(n, dtype=np.int64)
    k = v // n_core
    l = v % n_core
    b = l // P
    chb = np.asarray(chb, np.int64)
    c = np.searchsorted(chb, b, "right") - 1
    CR = (chb[1:] - chb[:-1]) * P
    crb = np.concatenate([[0], np.cumsum(CR)])[:-1]
    return 8 * crb[c] + k * CR[c] + (l - P * chb[c])

BF16 = mybir.dt.bfloat16
F32 = mybir.dt.float32
I16 = mybir.dt.int16
BF = ml_dtypes.bfloat16


def _prep_direction(dst, row, n_dst_core, nblk, nwin, wsize):
    """Edge metadata for one direction, pair-slot layout.

    Per (core, w, h, b) group: edges of each dst j are paired; full pairs
    (up to a min-over-cores pack budget) form 2-tile packs whose slots are
    pair-sums; leftovers go to raw 1-tile units (slot per edge). Returns
    (idx16 [CORES,128,Ttot*8], pdst [CORES,128,MUtot] bf16,
     npk2 [NG], nraw [NG]).
    """
    dst = np.asarray(dst, np.int64)
    row = np.asarray(row, np.int64)
    NE = dst.size

    w = row // wsize
    h = row % 2
    half = (row % wsize) // 2

    core = dst // n_dst_core
    dloc = dst % n_dst_core
    b = dloc // P
    j = dloc % P

    NG = nwin * 2 * nblk
    grp = (w * 2 + h) * nblk + b

    order = np.lexsort((half, j, grp, core))
    grp_s, core_s, half_s, j_s = grp[order], core[order], half[order], j[order]

    # rank within (core, grp, j) run
    rid = (core_s * NG + grp_s) * P + j_s
    rcnt = np.bincount(rid, minlength=CORES * NG * P)
    rstart = np.zeros(rcnt.size + 1, np.int64)
    rstart[1:] = np.cumsum(rcnt)
    rank = np.arange(NE, dtype=np.int64) - rstart[rid]
    nrun = rcnt[rid]
    is_pair = rank < (nrun - nrun % 2)

    # pair-slot number within (core, grp): pairs of earlier j-runs + own
    pj = rcnt // 2                                    # pairs per run
    pj_cg = pj.reshape(CORES * NG, P)
    pj_cum = np.cumsum(pj_cg, 1) - pj_cg              # pairs before run, in-grp
    pair_slot = pj_cum.reshape(-1)[rid] + rank // 2   # valid where is_pair
    elem = rank % 2

    S2 = pj_cg.sum(1).reshape(CORES, NG)              # pair slots per core/grp
    egg = rcnt.reshape(CORES * NG, P).sum(1).reshape(CORES, NG)
    # pick the pack budget q per group minimizing gather-idx + matmul cost;
    # under-budget cores auto-pad pack slots (pdst=-1 routes nothing)
    qlo = (S2 // P).min(0)
    qhi = -(-S2.max(0) // P)
    best_cost = None
    npk2 = qlo.copy()
    for q in range(0, int(qhi.max()) + 1):
        qv = np.minimum(np.maximum(qlo, q), qhi)
        packed = np.minimum(S2, qv[None, :] * P)
        rawe = egg - 2 * packed
        nrw = -(-rawe.max(0) // P)
        T = 2 * qv + nrw
        MUq = qv + nrw
        cost = 128 * T * 2.4 + MUq * 300.0
        if best_cost is None:
            best_cost = cost
        better = cost < best_cost
        npk2 = np.where(better, qv, npk2)
        best_cost = np.minimum(best_cost, cost)
    in_pack = is_pair & (pair_slot < npk2[grp_s] * P)

    # raw-slot rank per (core, grp) among non-packed edges, keep sort order
    kcg = core_s * NG + grp_s
    raw_mask = ~in_pack
    raw_rank = np.zeros(NE, np.int64)
    kraw = kcg[raw_mask]
    o2 = np.argsort(kraw, kind="stable")
    cnt_raw = np.bincount(kraw, minlength=CORES * NG)
    st = np.zeros(CORES * NG + 1, np.int64)
    st[1:] = np.cumsum(cnt_raw)
    rr = np.empty(kraw.size, np.int64)
    rr[o2] = np.arange(kraw.size) - st[kraw[o2]]
    raw_rank[raw_mask] = rr
    nraw = -(-cnt_raw.reshape(CORES, NG).max(0) // P)

    T_g = 2 * npk2 + nraw                             # tiles per grp
    MU_g = npk2 + nraw                                # matmul units per grp
    TB = np.cumsum(T_g) - T_g
    MB = np.cumsum(MU_g) - MU_g
    Ttot, MUtot = int(T_g.sum()), int(MU_g.sum())

    # flat idx position and (mu, slot) per edge
    pk = pair_slot // P
    ps = pair_slot % P
    tpos = np.where(in_pack,
                    (TB[grp_s] + 2 * pk + elem) * P + ps,
                    (TB[grp_s] + 2 * npk2[grp_s] + raw_rank // P) * P
                    + raw_rank % P)
    mu = np.where(in_pack, MB[grp_s] + pk,
                  MB[grp_s] + npk2[grp_s] + raw_rank // P)
    mslot = np.where(in_pack, ps, raw_rank % P)

    idx16 = np.zeros((CORES, Ttot * P), np.int16)
    valid = np.zeros((CORES, Ttot * P), bool)
    idx16[core_s, tpos] = half_s.astype(np.int16)
    valid[core_s, tpos] = True
    pdst = np.full((CORES, MUtot * P), -1.0, np.float32)
    pdst[core_s, mu * P + mslot] = j_s

    for k in range(CORES):
        v = valid[k]
        pos = np.where(v, np.arange(Ttot * P), 0)
        np.maximum.accumulate(pos, out=pos)
        idx16[k] = idx16[k][pos]

    packed = np.zeros((CORES, P, Ttot * 8), np.int16)
    for k in range(CORES):
        a = idx16[k].reshape(-1, 16).T
        packed[k] = np.tile(a, (8, 1))

    pdst_t = pdst.reshape(CORES, MUtot, P).transpose(0, 2, 1)
    return packed, np.ascontiguousarray(pdst_t).astype(BF), npk2, nraw


def _plan(npk2, nraw, nblk, nwin):
    """Compile-time schedule.

    Returns (calls, units, blk_groups): calls = (w, h, t0, mu0, unit_idx
    list); units[u] = (grp, is_pack, tile_off_in_grp, first_mu, last_mu).
    """
    NG = len(npk2)
    T_g = 2 * npk2 + nraw
    MU_g = npk2 + nraw
    TB = np.cumsum(T_g) - T_g
    MB = np.cumsum(MU_g) - MU_g
    units = []
    for g in range(NG):
        for k in range(int(npk2[g])):
            units.append((g, True, 2 * k, k == 0,
                          k == int(MU_g[g]) - 1))
        for r in range(int(nraw[g])):
            units.append((g, False, 2 * int(npk2[g]) + r,
                          int(npk2[g]) + r == 0,
                          int(npk2[g]) + r == int(MU_g[g]) - 1))
    calls = []
    for wh in range(nwin * 2):
        g0, g1 = wh * nblk, (wh + 1) * nblk
        u = int(MB[g0])
        u_end = int(MB[g1 - 1] + MU_g[g1 - 1])
        while u < u_end:
            nt = 0
            ulist = []
            while u < u_end and nt + (2 if units[u][1] else 1) <= TPC:
                ulist.append(u)
                nt += 2 if units[u][1] else 1
                u += 1
            g0t, _, off0, _, _ = units[ulist[0]]
            t0 = int(TB[g0t]) + off0
            calls.append((wh // 2, wh % 2, t0, ulist[0], ulist))
    blk_groups = [[] for _ in range(nblk)]
    for g in range(NG):
        if MU_g[g] > 0:
            blk_groups[g % nblk].append(g)
    return calls, units, blk_groups


def _dma_gather_raw(gp, out_ap, in_ap, idxs_ap, num_idxs, elem_size, elem_step,
                    queue_num=0):
    """dma_gather (non-transpose, HBM source) allowing 128B rows at 256B stride."""
    from concourse import ap_utils
    gp._assert_queue_num(queue_num)
    assert idxs_ap.dtype == mybir.dt.int16
    assert in_ap.dtype == out_ap.dtype
    assert ap_utils.ap_is_contiguous(in_ap.ap[1:])
    assert ap_utils.ap_is_contiguous(out_ap.ap[1:])
    assert ap_utils.ap_is_contiguous(idxs_ap.ap[1:])
    assert in_ap.ap[-1][1] == out_ap.ap[-1][1] == elem_size
    assert out_ap.ap[0][1] * out_ap.ap[1][1] == num_idxs and num_idxs % 128 == 0
    assert in_ap.ap[0][0] == elem_step
    stride_bytes = elem_step * mybir.dt.size(in_ap.dtype)
    stride_bytes_256 = stride_bytes // 256
    assert stride_bytes_256 * 256 == stride_bytes and stride_bytes_256 < 256
    _in_ap = gp.lower_ap_dma(in_ap, for_custom_bir_dma=True)
    _idxs_ap = gp.lower_ap(idxs_ap)
    _out_ap = gp.lower_ap(out_ap)
    return gp.add_instruction(
        mybir.InstDMAGatherAnt(
            name=gp.bass.get_next_instruction_name(),
            ins=[*_in_ap, _idxs_ap, gp.lower_val_access(gp.to_reg(num_idxs))],
            outs=[_out_ap],
            transpose=False, num_idxs=num_idxs, elem_size=elem_size,
            stride_bytes_256=stride_bytes_256, gen_mode=0, single_packet=True,
            queue_num=queue_num, sbuf_tokens_per_rank=0,
            sbuf_free_dim_per_rank=0, sbuf_free_dim_pad_per_rank=0,
            sbuf_byte_offset=0))


def _build(meta_c, meta_v, b_score_val):
    npk2_c, nraw_c = meta_c
    npk2_v, nraw_v = meta_v
    Tt_c = int((2 * npk2_c + nraw_c).sum())
    Tt_v = int((2 * npk2_v + nraw_v).sum())
    Mt_c = int((npk2_c + nraw_c).sum())
    Mt_v = int((npk2_v + nraw_v).sum())
    calls_c, units_c, bg_c = _plan(npk2_c, nraw_c, NB_C, VWIN)
    calls_v, units_v, bg_v = _plan(npk2_v, nraw_v, NB_V, CWIN)

    nc = bacc.Bacc("TRN2", target_bir_lowering=False, num_devices=CORES,
                   num_swdge_queues=4)
    AluOp = mybir.AluOpType
    Act = mybir.ActivationFunctionType

    def ein(name, shape, dtype):
        return nc.dram_tensor(name, shape, dtype, kind="ExternalInput")

    vfT = ein("vfT", [VF, V_S], F32)
    cfT = ein("cfT", [CF, C_S], F32)
    wvar = ein("wvar", [VF, H], F32)
    wcon = ein("wcon", [CF, H], F32)
    wv2c = ein("wv2c", [H, H], F32)
    wc2v = ein("wc2v", [H, H], F32)
    wsco = ein("wsco", [H, 1], BF16)
    bvar = ein("bvar", [H, 1], F32)
    bcon = ein("bcon", [H, 1], F32)
    bv2c = ein("bv2c", [H, 1], F32)
    bc2v = ein("bc2v", [H, 1], F32)
    idx_v2c_d = ein("idx_v2c", [P, Tt_c * 8], I16)
    idx_c2v_d = ein("idx_c2v", [P, Tt_v * 8], I16)
    pdst_v2c_d = ein("pdst_v2c", [P, Mt_c], BF16)
    pdst_c2v_d = ein("pdst_c2v", [P, Mt_v], BF16)
    iota_d = ein("iota", [P, P], BF16)
    ident_d = ein("ident", [H, H], BF16)
    scores_out = nc.dram_tensor("scores", [V_S], F32, kind="ExternalOutput")

    with tile.TileContext(nc) as tc:
        with (
            tc.tile_pool(name="const", bufs=1) as cpool,
            tc.tile_pool(name="state", bufs=1) as spool,
            tc.tile_pool(name="dram", bufs=1, space="DRAM") as dpool,
            tc.tile_pool(name="gpool", bufs=24) as gpool,
            tc.tile_pool(name="s_pool", bufs=10) as s_pool,
            tc.tile_pool(name="misc", bufs=4) as mpool,
            tc.tile_pool(name="ps_acc", bufs=4, space="PSUM") as ps_acc,
            tc.tile_pool(name="ps_upd", bufs=2, space="PSUM") as ps_upd,
            tc.tile_pool(name="ps_misc", bufs=2, space="PSUM") as ps_misc,
        ):
            def load_const(name, dram, shape, dtype):
                t = cpool.tile(shape, dtype, name=name)
                nc.sync.dma_start(out=t[:], in_=dram[:])
                return t

            iota_sb = load_const("iota_sb", iota_d, [P, P], BF16)
            ident_sb = load_const("ident_sb", ident_d, [H, H], BF16)
            wvar_sb = load_const("wvar_sb", wvar, [VF, H], F32)
            wcon_sb = load_const("wcon_sb", wcon, [CF, H], F32)
            wv2c_sb = load_const("wv2c_sb", wv2c, [H, H], F32)
            wc2v_sb = load_const("wc2v_sb", wc2v, [H, H], F32)
            wsco_sb = load_const("wsco_sb", wsco, [H, 1], BF16)
            bvar_sb = load_const("bvar_sb", bvar, [H, 1], F32)
            bcon_sb = load_const("bcon_sb", bcon, [H, 1], F32)
            bv2c_sb = load_const("bv2c_sb", bv2c, [H, 1], F32)
            bc2v_sb = load_const("bc2v_sb", bc2v, [H, 1], F32)
            idx_c_sb = load_const("idx_c_sb", idx_v2c_d, [P, Tt_c * 8], I16)
            idx_v_sb = load_const("idx_v_sb", idx_c2v_d, [P, Tt_v * 8], I16)
            pdst_c_sb = load_const("pdst_c_sb", pdst_v2c_d, [P, Mt_c], BF16)
            pdst_v_sb = load_const("pdst_v_sb", pdst_c2v_d, [P, Mt_v], BF16)

            hvT = spool.tile([H, V_S], BF16, name="hvT")
            hcT = spool.tile([H, C_S], BF16, name="hcT")
            macc = spool.tile([H, NB_V * P], F32, name="macc")
            rstage = spool.tile([P, NB_V * H], BF16, name="rstage")

            tabs_v = [[dpool.tile([RV // VWIN, H], BF16, name=f"tab_v{i}_{w}",
                                  addr_space="Shared", tag=f"tab_v{i}_{w}")
                       for w in range(VWIN)] for i in range(ROUNDS)]
            tabs_c = [[dpool.tile([RC // CWIN, H], BF16, name=f"tab_c{i}_{w}",
                                  addr_space="Shared", tag=f"tab_c{i}_{w}")
                       for w in range(CWIN)] for i in range(ROUNDS)]
            agin_v = dpool.tile([V_S, H], BF16, name="agin_v")
            agin_c = dpool.tile([C_S, H], BF16, name="agin_c")

            # ---- initial embeddings hT = relu(W.T @ featT + b)
            def emit_init(featT_dram, fdim, n_s, w_sb, b_sb, hT):
                with tc.tile_pool(name="initp", bufs=2) as ipool:
                    c0 = 0
                    while c0 < n_s:
                        wd = min(512, n_s - c0)
                        fch = ipool.tile([fdim, 512], F32, name="fch", tag="fch")
                        nc.sync.dma_start(out=fch[:, :wd],
                                          in_=featT_dram[:, c0:c0 + wd])
                        psi = ps_misc.tile([H, 512], F32, name="psi", tag="misc")
                        nc.tensor.matmul(out=psi[:, :wd], lhsT=w_sb[:],
                                         rhs=fch[:, :wd], start=True, stop=True)
                        nc.scalar.activation(out=hT[:, c0:c0 + wd], in_=psi[:, :wd],
                                             func=Act.Relu, bias=b_sb[:])
                        c0 += wd

            emit_init(vfT, VF, V_S, wvar_sb, bvar_sb, hvT)

            def emit_chunk_dma(agin, b0, b1):
                nc.sync.dma_start(
                    out=agin[b0 * P:b1 * P, :].rearrange("(b p) f -> p b f", p=P),
                    in_=rstage[:, b0 * H:b1 * H].rearrange("p (b f) -> p b f",
                                                           f=H))

            def emit_chunk_colls(agin, tabs, coll_chb):
                for c in range(len(coll_chb) - 1):
                    b0, b1 = coll_chb[c], coll_chb[c + 1]
                    nc.gpsimd.collective_compute(
                        "AllGather", mybir.AluOpType.bypass,
                        replica_groups=[list(range(CORES))],
                        ins=[agin[b0 * P:b1 * P, :]],
                        outs=[tabs[c][:]])

            def emit_writeback(hT, nblk, agin, tabs, coll_chb, dma_chb):
                for b in range(nblk):
                    psr = ps_misc.tile([P, H], BF16, name="psr", tag="misc")
                    nc.tensor.transpose(out=psr[:], in_=hT[:, b * P:(b + 1) * P],
                                        identity=ident_sb[:])
                    nc.vector.tensor_copy(out=rstage[:, b * H:(b + 1) * H],
                                          in_=psr[:])
                for c in range(len(dma_chb) - 1):
                    emit_chunk_dma(agin, dma_chb[c], dma_chb[c + 1])
                emit_chunk_colls(agin, tabs, coll_chb)

            emit_writeback(hvT, NB_V, agin_v, tabs_v[0], VCH, VDM)
            emit_init(cfT, CF, C_S, wcon_sb, bcon_sb, hcT)

            # ---- one message-passing phase
            def emit_phase(tab_srcs, nwin, idx_sb, pdst_sb,
                           calls, units, blk_groups, nblk, hT, W_sb, b_sb,
                           wb):
                # even/odd row views of the per-window table tensors
                win_ap = {}
                for w in range(nwin):
                    tab2 = tab_srcs[w][:].rearrange("(n two) f -> n (two f)",
                                                    two=2)
                    for h in range(2):
                        win_ap[(w, h)] = tab2[:, h * H:(h + 1) * H]

                accs = {}
                done_groups = [0] * nblk
                if wb is not None:
                    agin, tabs, coll_chb, chb = wb
                    chunk_left = [chb[c + 1] - chb[c]
                                  for c in range(len(chb) - 1)]

                def emit_update(b):
                    ps2 = ps_upd.tile([H, P], F32, name="ps2", tag="ps2")
                    nc.tensor.matmul(out=ps2[:], lhsT=W_sb[:],
                                     rhs=macc[:, b * P:(b + 1) * P],
                                     start=True, stop=True)
                    tmp = mpool.tile([H, P], F32, name="tmp", tag="tmp")
                    nc.vector.tensor_tensor(out=tmp[:], in0=ps2[:],
                                            in1=hT[:, b * P:(b + 1) * P],
                                            op=AluOp.add)
                    nc.scalar.activation(out=hT[:, b * P:(b + 1) * P],
                                         in_=tmp[:], func=Act.Relu, bias=b_sb[:])
                    if wb is not None:
                        psr = ps_misc.tile([P, H], BF16, name="psr", tag="misc")
                        nc.tensor.transpose(out=psr[:],
                                            in_=hT[:, b * P:(b + 1) * P],
                                            identity=ident_sb[:])
                        nc.scalar.activation(
                            out=rstage[:, b * H:(b + 1) * H], in_=psr[:],
                            func=Act.Identity)
                        c = int(np.searchsorted(chb, b, "right")) - 1
                        chunk_left[c] -= 1
                        if chunk_left[c] == 0:
                            emit_chunk_dma(agin, chb[c], chb[c + 1])

                for ci, (w, h, t0, mu0, ulist) in enumerate(calls):
                    nt = sum(2 if units[u][1] else 1 for u in ulist)
                    nmu = len(ulist)
                    g = gpool.tile([P, TPC, H], BF16, name="g", tag="g")
                    _dma_gather_raw(
                        nc.gpsimd, g[:, :nt, :], win_ap[(w, h)],
                        idx_sb[:, t0 * 8:(t0 + nt) * 8],
                        num_idxs=nt * P, elem_size=H, elem_step=2 * H,
                        queue_num=ci % 4)
                    S = s_pool.tile([P, TPC, P], BF16, name="S", tag="S")
                    nc.vector.tensor_tensor(
                        out=S[:, :nmu, :],
                        in0=iota_sb[:, None, :].to_broadcast([P, nmu, P]),
                        in1=pdst_sb[:, mu0:mu0 + nmu, None]
                            .to_broadcast([P, nmu, P]),
                        op=AluOp.is_equal)
                    off = 0
                    for mi, u in enumerate(ulist):
                        grp, is_pack, _, first, last = units[u]
                        if is_pack:
                            ss = mpool.tile([P, H], BF16, name="ss", tag="ss",
                                            bufs=4)
                            nc.vector.tensor_tensor(
                                out=ss[:], in0=g[:, off, :], in1=g[:, off + 1, :],
                                op=AluOp.add)
                            lhs = ss[:]
                            off += 2
                        else:
                            lhs = g[:, off, :]
                            off += 1
                        if first:
                            accs[grp] = ps_acc.tile([H, P], F32, name="acc",
                                                    tag="acc")
                        nc.tensor.matmul(out=accs[grp][:], lhsT=lhs,
                                         rhs=S[:, mi, :], start=first, stop=last)
                        if last:
                            b = grp % nblk
                            glist = blk_groups[b]
                            if done_groups[b] == 0:
                                nc.scalar.activation(
                                    out=macc[:, b * P:(b + 1) * P],
                                    in_=accs[grp][:], func=Act.Identity)
                            else:
                                nc.vector.tensor_tensor(
                                    out=macc[:, b * P:(b + 1) * P],
                                    in0=macc[:, b * P:(b + 1) * P],
                                    in1=accs[grp][:], op=AluOp.add)
                            del accs[grp]
                            done_groups[b] += 1
                            if done_groups[b] == len(glist):
                                emit_update(b)
                if wb is not None:
                    emit_chunk_colls(agin, tabs, coll_chb)

            for r in range(ROUNDS):
                emit_phase(tabs_v[r], VWIN, idx_c_sb, pdst_c_sb,
                           calls_c, units_c, bg_c, NB_C, hcT, wv2c_sb, bv2c_sb,
                           (agin_c, tabs_c[r], CCH, CDM))
                last = r == ROUNDS - 1
                emit_phase(tabs_c[r], CWIN, idx_v_sb, pdst_v_sb,
                           calls_v, units_v, bg_v, NB_V, hvT, wc2v_sb, bc2v_sb,
                           None if last else
                           (agin_v, tabs_v[r + 1], VCH, VDM))

            # ---- scores = h_var @ w_score + b_score (shard)
            c0 = 0
            while c0 < V_S:
                wd = min(512, V_S - c0)
                pss = ps_misc.tile([1, 512], F32, name="pss", tag="misc")
                nc.tensor.matmul(out=pss[:, :wd], lhsT=wsco_sb[:],
                                 rhs=hvT[:, c0:c0 + wd], start=True, stop=True)
                sch = mpool.tile([1, 512], F32, name="sch", tag="sch")
                nc.vector.tensor_scalar(
                    out=sch[:, :wd], in0=pss[:, :wd],
                    scalar1=float(b_score_val), scalar2=None, op0=AluOp.add)
                nc.sync.dma_start(out=scores_out[None, c0:c0 + wd],
                                  in_=sch[0:1, :wd])
                c0 += wd

    nc.compile()
    return nc


_CACHE = {}


def kernel(**inputs):
    var_feat = np.asarray(inputs["var_feat"], np.float32)
    constr_feat = np.asarray(inputs["constr_feat"], np.float32)
    var_idx = np.asarray(inputs["var_idx"]).astype(np.int64)
    constr_idx = np.asarray(inputs["constr_idx"]).astype(np.int64)
    b_score_val = float(np.asarray(inputs["b_score"]).reshape(-1)[0])

    key = (var_idx.tobytes(), constr_idx.tobytes())
    if key in _CACHE:
        nc, idx_v, pdst_v, idx_c, pdst_c = _CACHE[key]
    else:
        rm_v = _row_map(V_CORE, VCH)
        rm_c = _row_map(C_CORE, CCH)
        # v2c: dst=constr, src=var
        idx_v, pdst_v, npk2_c, nraw_c = _prep_direction(
            constr_idx, rm_v[var_idx], C_CORE, NB_C, VWIN, RV // VWIN)
        # c2v: dst=var, src=constr
        idx_c, pdst_c, npk2_v, nraw_v = _prep_direction(
            var_idx, rm_c[constr_idx], V_CORE, NB_V, CWIN, RC // CWIN)
        nc = _build((npk2_c, nraw_c), (npk2_v, nraw_v), b_score_val)
        _CACHE[key] = (nc, idx_v, pdst_v, idx_c, pdst_c)

    iota = np.broadcast_to(np.arange(P, dtype=np.float32),
                           (P, P)).astype(BF).copy()
    ident = np.eye(H, dtype=np.float32).astype(BF)

    vf_pad = np.zeros((CORES, V_S, VF), np.float32)
    vf_pad[:, :V_CORE] = var_feat.reshape(CORES, V_CORE, VF)
    cf_pad = np.zeros((CORES, C_S, CF), np.float32)
    cf_pad[:, :C_CORE] = constr_feat.reshape(CORES, C_CORE, CF)

    common = dict(
        wvar=np.ascontiguousarray(inputs["W_var"], dtype=np.float32),
        wcon=np.ascontiguousarray(inputs["W_con"], dtype=np.float32),
        wv2c=np.ascontiguousarray(inputs["W_v2c"], dtype=np.float32),
        wc2v=np.ascontiguousarray(inputs["W_c2v"], dtype=np.float32),
        wsco=np.ascontiguousarray(inputs["W_score"], dtype=np.float32).astype(BF),
        bvar=np.ascontiguousarray(inputs["b_var"], dtype=np.float32).reshape(H, 1),
        bcon=np.ascontiguousarray(inputs["b_con"], dtype=np.float32).reshape(H, 1),
        bv2c=np.ascontiguousarray(inputs["b_v2c"], dtype=np.float32).reshape(H, 1),
        bc2v=np.ascontiguousarray(inputs["b_c2v"], dtype=np.float32).reshape(H, 1),
        iota=iota, ident=ident,
    )
    in_maps = []
    for k in range(CORES):
        m = dict(common)
        m["vfT"] = np.ascontiguousarray(vf_pad[k].T)
        m["cfT"] = np.ascontiguousarray(cf_pad[k].T)
        m["idx_v2c"] = idx_v[k]
        m["pdst_v2c"] = pdst_v[k]
        m["idx_c2v"] = idx_c[k]
        m["pdst_c2v"] = pdst_c[k]
        in_maps.append(m)

    res = run_bass_kernel_spmd(nc, in_maps, list(range(CORES)))
    scores = np.concatenate([res.results[k]["scores"].reshape(-1)[:V_CORE]
                             for k in range(CORES)])
    return scores.astype(np.float32)


# revision 37
# speedup vs baseline: 1.1919x; 1.1919x over previous
"""Trainium2 Bass kernel: bipartite GNN message passing (BranchingGNN), 8-core SPMD.

Sharding: core k owns constraint rows [k*6250,(k+1)*6250) and variable rows
[k*12500,(k+1)*12500); each core processes all edges targeting its shard, so
messages need no cross-core reduction. Node tables are compact row-major
[N, 64] bf16 in DRAM and are re-broadcast each phase by an AllGather of the
updated shard.

Per phase (one message direction):
  - edges sorted by (src-window, src-parity, dst-block, src-row); each
    (window,parity,block) group is padded to 128-edge tiles. Source rows are
    gathered by dma_gather (128B rows at 256B stride over the even/odd row
    subsequence of the compact table), ascending addresses for HBM locality.
  - per tile, a one-hot S [128e,128d] = (iota == dstcol) is built on DVE
    (batched per gather call); one PE matmul per tile accumulates
    msgT [64,128] into a per-group PSUM tile (start/stop over the group).
  - group partials are added into an SBUF accumulator macc per dst block;
    the block update relu(hT + W.T @ msgT + b) runs as soon as its last
    group lands; PE transpose + one DMA + AllGather republish the table.
"""
import sys

sys.path.insert(0, "/opt/trn_rl_repo")

import numpy as np
import ml_dtypes

import concourse.bass as bass
import concourse.bacc as bacc
import concourse.mybir as mybir
import concourse.tile as tile
from concourse.bass_utils import run_bass_kernel_spmd

# ---- problem constants
V, C, E = 100000, 50000, 1250000
VF, CF, H = 32, 32, 64
ROUNDS = 3
CORES = 8
P = 128
TPC = 7               # tiles per gather call (ring cap 64 descs)

V_CORE, C_CORE = 12500, 6250          # real nodes per core
V_S, C_S = 12544, 6272                # shard rows (98 / 49 blocks)
NB_V, NB_C = 98, 49                   # dst blocks per core
RV, RC = CORES * V_S, CORES * C_S     # 100352 / 50176 table rows
VWIN, CWIN = 2, 1                     # source windows (half-row reach 32767)
# chunk-major table layout: block ranges per table tensor (= gather window,
# = one AllGather each); plus finer DMA staging chunks
VCH = [0, 49, 98]
CCH = [0, 49]
VDM = [0, 25, 49, 74, 98]
CDM = [0, 13, 25, 37, 49]


def _row_map(n_core, chb):
    """node id -> chunk-major table row."""
    n = n_core * CORES
    v = np.arange(n, dtype=np.int64)
    k = v // n_core
    l = v % n_core
    b = l // P
    chb = np.asarray(chb, np.int64)
    c = np.searchsorted(chb, b, "right") - 1
    CR = (chb[1:] - chb[:-1]) * P
    crb = np.concatenate([[0], np.cumsum(CR)])[:-1]
    return 8 * crb[c] + k * CR[c] + (l - P * chb[c])

BF16 = mybir.dt.bfloat16
F32 = mybir.dt.float32
I16 = mybir.dt.int16
BF = ml_dtypes.bfloat16


def _prep_direction(dst, row, n_dst_core, nblk, nwin, wsize):
    """Edge metadata for one direction, pair-slot layout.

    Per (core, w, h, b) group: edges of each dst j are paired; full pairs
    (up to a min-over-cores pack budget) form 2-tile packs whose slots are
    pair-sums; leftovers go to raw 1-tile units (slot per edge). Returns
    (idx16 [CORES,128,Ttot*8], pdst [CORES,128,MUtot] bf16,
     npk2 [NG], nraw [NG]).
    """
    dst = np.asarray(dst, np.int64)
    row = np.asarray(row, np.int64)
    NE = dst.size

    w = row // wsize
    h = row % 2
    half = (row % wsize) // 2

    core = dst // n_dst_core
    dloc = dst % n_dst_core
    b = dloc // P
    j = dloc % P

    NG = nwin * 2 * nblk
    grp = (w * 2 + h) * nblk + b

    order = np.lexsort((half, j, grp, core))
    grp_s, core_s, half_s, j_s = grp[order], core[order], half[order], j[order]

    # rank within (core, grp, j) run
    rid = (core_s * NG + grp_s) * P + j_s
    rcnt = np.bincount(rid, minlength=CORES * NG * P)
    rstart = np.zeros(rcnt.size + 1, np.int64)
    rstart[1:] = np.cumsum(rcnt)
    rank = np.arange(NE, dtype=np.int64) - rstart[rid]
    nrun = rcnt[rid]
    is_pair = rank < (nrun - nrun % 2)

    # pair-slot number within (core, grp): pairs of earlier j-runs + own
    pj = rcnt // 2                                    # pairs per run
    pj_cg = pj.reshape(CORES * NG, P)
    pj_cum = np.cumsum(pj_cg, 1) - pj_cg              # pairs before run, in-grp
    pair_slot = pj_cum.reshape(-1)[rid] + rank // 2   # valid where is_pair
    elem = rank % 2

    S2 = pj_cg.sum(1).reshape(CORES, NG)              # pair slots per core/grp
    egg = rcnt.reshape(CORES * NG, P).sum(1).reshape(CORES, NG)
    # pick the pack budget q per group minimizing gather-idx + matmul cost;
    # under-budget cores auto-pad pack slots (pdst=-1 routes nothing)
    qlo = (S2 // P).min(0)
    qhi = -(-S2.max(0) // P)
    best_cost = None
    npk2 = qlo.copy()
    for q in range(0, int(qhi.max()) + 1):
        qv = np.minimum(np.maximum(qlo, q), qhi)
        packed = np.minimum(S2, qv[None, :] * P)
        rawe = egg - 2 * packed
        nrw = -(-rawe.max(0) // P)
        T = 2 * qv + nrw
        MUq = qv + nrw
        cost = 128 * T * 2.4 + MUq * 300.0
        if best_cost is None:
            best_cost = cost
        better = cost < best_cost
        npk2 = np.where(better, qv, npk2)
        best_cost = np.minimum(best_cost, cost)
    in_pack = is_pair & (pair_slot < npk2[grp_s] * P)

    # raw-slot rank per (core, grp) among non-packed edges, keep sort order
    kcg = core_s * NG + grp_s
    raw_mask = ~in_pack
    raw_rank = np.zeros(NE, np.int64)
    kraw = kcg[raw_mask]
    o2 = np.argsort(kraw, kind="stable")
    cnt_raw = np.bincount(kraw, minlength=CORES * NG)
    st = np.zeros(CORES * NG + 1, np.int64)
    st[1:] = np.cumsum(cnt_raw)
    rr = np.empty(kraw.size, np.int64)
    rr[o2] = np.arange(kraw.size) - st[kraw[o2]]
    raw_rank[raw_mask] = rr
    nraw = -(-cnt_raw.reshape(CORES, NG).max(0) // P)

    T_g = 2 * npk2 + nraw                             # tiles per grp
    MU_g = npk2 + nraw                                # matmul units per grp
    TB = np.cumsum(T_g) - T_g
    MB = np.cumsum(MU_g) - MU_g
    Ttot, MUtot = int(T_g.sum()), int(MU_g.sum())

    # flat idx position and (mu, slot) per edge
    pk = pair_slot // P
    ps = pair_slot % P
    tpos = np.where(in_pack,
                    (TB[grp_s] + 2 * pk + elem) * P + ps,
                    (TB[grp_s] + 2 * npk2[grp_s] + raw_rank // P) * P
                    + raw_rank % P)
    mu = np.where(in_pack, MB[grp_s] + pk,
                  MB[grp_s] + npk2[grp_s] + raw_rank // P)
    mslot = np.where(in_pack, ps, raw_rank % P)

    idx16 = np.zeros((CORES, Ttot * P), np.int16)
    valid = np.zeros((CORES, Ttot * P), bool)
    idx16[core_s, tpos] = half_s.astype(np.int16)
    valid[core_s, tpos] = True
    pdst = np.full((CORES, MUtot * P), -1.0, np.float32)
    pdst[core_s, mu * P + mslot] = j_s

    for k in range(CORES):
        v = valid[k]
        pos = np.where(v, np.arange(Ttot * P), 0)
        np.maximum.accumulate(pos, out=pos)
        idx16[k] = idx16[k][pos]

    packed = np.zeros((CORES, P, Ttot * 8), np.int16)
    for k in range(CORES):
        a = idx16[k].reshape(-1, 16).T
        packed[k] = np.tile(a, (8, 1))

    pdst_t = pdst.reshape(CORES, MUtot, P).transpose(0, 2, 1)
    return packed, np.ascontiguousarray(pdst_t).astype(BF), npk2, nraw


def _plan(npk2, nraw, nblk, nwin):
    """Compile-time schedule.

    Returns (calls, units, blk_groups): calls = (w, h, t0, mu0, unit_idx
    list); units[u] = (grp, is_pack, tile_off_in_grp, first_mu, last_mu).
    """
    NG = len(npk2)
    T_g = 2 * npk2 + nraw
    MU_g = npk2 + nraw
    TB = np.cumsum(T_g) - T_g
    MB = np.cumsum(MU_g) - MU_g
    units = []
    for g in range(NG):
        for k in range(int(npk2[g])):
            units.append((g, True, 2 * k, k == 0,
                          k == int(MU_g[g]) - 1))
        for r in range(int(nraw[g])):
            units.append((g, False, 2 * int(npk2[g]) + r,
                          int(npk2[g]) + r == 0,
                          int(npk2[g]) + r == int(MU_g[g]) - 1))
    # calls chunk 7 tiles regardless of unit boundaries; a pack may span two
    # calls (its pair-add reads from two g buffers). Each call carries the
    # units whose LAST tile lands in it.
    calls = []
    u = 0
    for wh in range(nwin * 2):
        g0, g1 = wh * nblk, (wh + 1) * nblk
        t = int(TB[g0])
        t_end = int(TB[g1 - 1] + T_g[g1 - 1])
        while t < t_end:
            nt = min(TPC, t_end - t)
            ulist = []
            while u < len(units):
                grp, is_pack, off, _, _ = units[u]
                lastt = int(TB[grp]) + off + (1 if is_pack else 0)
                if lastt >= t + nt:
                    break
                ulist.append(u)
                u += 1
            calls.append((wh // 2, wh % 2, t, nt, ulist))
            t += nt
    blk_groups = [[] for _ in range(nblk)]
    for g in range(NG):
        if MU_g[g] > 0:
            blk_groups[g % nblk].append(g)
    return calls, units, blk_groups, TB


def _dma_gather_raw(gp, out_ap, in_ap, idxs_ap, num_idxs, elem_size, elem_step,
                    queue_num=0):
    """dma_gather (non-transpose, HBM source) allowing 128B rows at 256B stride."""
    from concourse import ap_utils
    gp._assert_queue_num(queue_num)
    assert idxs_ap.dtype == mybir.dt.int16
    assert in_ap.dtype == out_ap.dtype
    assert ap_utils.ap_is_contiguous(in_ap.ap[1:])
    assert ap_utils.ap_is_contiguous(out_ap.ap[1:])
    assert ap_utils.ap_is_contiguous(idxs_ap.ap[1:])
    assert in_ap.ap[-1][1] == out_ap.ap[-1][1] == elem_size
    assert out_ap.ap[0][1] * out_ap.ap[1][1] == num_idxs and num_idxs % 128 == 0
    assert in_ap.ap[0][0] == elem_step
    stride_bytes = elem_step * mybir.dt.size(in_ap.dtype)
    stride_bytes_256 = stride_bytes // 256
    assert stride_bytes_256 * 256 == stride_bytes and stride_bytes_256 < 256
    _in_ap = gp.lower_ap_dma(in_ap, for_custom_bir_dma=True)
    _idxs_ap = gp.lower_ap(idxs_ap)
    _out_ap = gp.lower_ap(out_ap)
    return gp.add_instruction(
        mybir.InstDMAGatherAnt(
            name=gp.bass.get_next_instruction_name(),
            ins=[*_in_ap, _idxs_ap, gp.lower_val_access(gp.to_reg(num_idxs))],
            outs=[_out_ap],
            transpose=False, num_idxs=num_idxs, elem_size=elem_size,
            stride_bytes_256=stride_bytes_256, gen_mode=0, single_packet=True,
            queue_num=queue_num, sbuf_tokens_per_rank=0,
            sbuf_free_dim_per_rank=0, sbuf_free_dim_pad_per_rank=0,
            sbuf_byte_offset=0))


def _build(meta_c, meta_v, b_score_val):
    npk2_c, nraw_c = meta_c
    npk2_v, nraw_v = meta_v
    Tt_c = int((2 * npk2_c + nraw_c).sum())
    Tt_v = int((2 * npk2_v + nraw_v).sum())
    Mt_c = int((npk2_c + nraw_c).sum())
    Mt_v = int((npk2_v + nraw_v).sum())
    calls_c, units_c, bg_c, TB_c = _plan(npk2_c, nraw_c, NB_C, VWIN)
    calls_v, units_v, bg_v, TB_v = _plan(npk2_v, nraw_v, NB_V, CWIN)

    nc = bacc.Bacc("TRN2", target_bir_lowering=False, num_devices=CORES,
                   num_swdge_queues=4)
    AluOp = mybir.AluOpType
    Act = mybir.ActivationFunctionType

    def ein(name, shape, dtype):
        return nc.dram_tensor(name, shape, dtype, kind="ExternalInput")

    vfT = ein("vfT", [VF, V_S], F32)
    cfT = ein("cfT", [CF, C_S], F32)
    wvar = ein("wvar", [VF, H], F32)
    wcon = ein("wcon", [CF, H], F32)
    wv2c = ein("wv2c", [H, H], F32)
    wc2v = ein("wc2v", [H, H], F32)
    wsco = ein("wsco", [H, 1], BF16)
    bvar = ein("bvar", [H, 1], F32)
    bcon = ein("bcon", [H, 1], F32)
    bv2c = ein("bv2c", [H, 1], F32)
    bc2v = ein("bc2v", [H, 1], F32)
    idx_v2c_d = ein("idx_v2c", [P, Tt_c * 8], I16)
    idx_c2v_d = ein("idx_c2v", [P, Tt_v * 8], I16)
    pdst_v2c_d = ein("pdst_v2c", [P, Mt_c], BF16)
    pdst_c2v_d = ein("pdst_c2v", [P, Mt_v], BF16)
    iota_d = ein("iota", [P, P], BF16)
    ident_d = ein("ident", [H, H], BF16)
    scores_out = nc.dram_tensor("scores", [V_S], F32, kind="ExternalOutput")

    with tile.TileContext(nc) as tc:
        with (
            tc.tile_pool(name="const", bufs=1) as cpool,
            tc.tile_pool(name="state", bufs=1) as spool,
            tc.tile_pool(name="dram", bufs=1, space="DRAM") as dpool,
            tc.tile_pool(name="gpool", bufs=24) as gpool,
            tc.tile_pool(name="s_pool", bufs=10) as s_pool,
            tc.tile_pool(name="misc", bufs=4) as mpool,
            tc.tile_pool(name="ps_acc", bufs=4, space="PSUM") as ps_acc,
            tc.tile_pool(name="ps_upd", bufs=2, space="PSUM") as ps_upd,
            tc.tile_pool(name="ps_misc", bufs=2, space="PSUM") as ps_misc,
        ):
            def load_const(name, dram, shape, dtype):
                t = cpool.tile(shape, dtype, name=name)
                nc.sync.dma_start(out=t[:], in_=dram[:])
                return t

            iota_sb = load_const("iota_sb", iota_d, [P, P], BF16)
            ident_sb = load_const("ident_sb", ident_d, [H, H], BF16)
            wvar_sb = load_const("wvar_sb", wvar, [VF, H], F32)
            wcon_sb = load_const("wcon_sb", wcon, [CF, H], F32)
            wv2c_sb = load_const("wv2c_sb", wv2c, [H, H], F32)
            wc2v_sb = load_const("wc2v_sb", wc2v, [H, H], F32)
            wsco_sb = load_const("wsco_sb", wsco, [H, 1], BF16)
            bvar_sb = load_const("bvar_sb", bvar, [H, 1], F32)
            bcon_sb = load_const("bcon_sb", bcon, [H, 1], F32)
            bv2c_sb = load_const("bv2c_sb", bv2c, [H, 1], F32)
            bc2v_sb = load_const("bc2v_sb", bc2v, [H, 1], F32)
            idx_c_sb = load_const("idx_c_sb", idx_v2c_d, [P, Tt_c * 8], I16)
            idx_v_sb = load_const("idx_v_sb", idx_c2v_d, [P, Tt_v * 8], I16)
            pdst_c_sb = load_const("pdst_c_sb", pdst_v2c_d, [P, Mt_c], BF16)
            pdst_v_sb = load_const("pdst_v_sb", pdst_c2v_d, [P, Mt_v], BF16)

            hvT = spool.tile([H, V_S], BF16, name="hvT")
            hcT = spool.tile([H, C_S], BF16, name="hcT")
            macc = spool.tile([H, NB_V * P], F32, name="macc")
            rstage = spool.tile([P, NB_V * H], BF16, name="rstage")

            tabs_v = [[dpool.tile([RV // VWIN, H], BF16, name=f"tab_v{i}_{w}",
                                  addr_space="Shared", tag=f"tab_v{i}_{w}")
                       for w in range(VWIN)] for i in range(ROUNDS)]
            tabs_c = [[dpool.tile([RC // CWIN, H], BF16, name=f"tab_c{i}_{w}",
                                  addr_space="Shared", tag=f"tab_c{i}_{w}")
                       for w in range(CWIN)] for i in range(ROUNDS)]
            agin_v = dpool.tile([V_S, H], BF16, name="agin_v")
            agin_c = dpool.tile([C_S, H], BF16, name="agin_c")

            # ---- initial embeddings hT = relu(W.T @ featT + b)
            def emit_init(featT_dram, fdim, n_s, w_sb, b_sb, hT):
                with tc.tile_pool(name="initp", bufs=2) as ipool:
                    c0 = 0
                    while c0 < n_s:
                        wd = min(512, n_s - c0)
                        fch = ipool.tile([fdim, 512], F32, name="fch", tag="fch")
                        nc.sync.dma_start(out=fch[:, :wd],
                                          in_=featT_dram[:, c0:c0 + wd])
                        psi = ps_misc.tile([H, 512], F32, name="psi", tag="misc")
                        nc.tensor.matmul(out=psi[:, :wd], lhsT=w_sb[:],
                                         rhs=fch[:, :wd], start=True, stop=True)
                        nc.scalar.activation(out=hT[:, c0:c0 + wd], in_=psi[:, :wd],
                                             func=Act.Relu, bias=b_sb[:])
                        c0 += wd

            emit_init(vfT, VF, V_S, wvar_sb, bvar_sb, hvT)

            def emit_chunk_dma(agin, b0, b1):
                nc.sync.dma_start(
                    out=agin[b0 * P:b1 * P, :].rearrange("(b p) f -> p b f", p=P),
                    in_=rstage[:, b0 * H:b1 * H].rearrange("p (b f) -> p b f",
                                                           f=H))

            def emit_chunk_colls(agin, tabs, coll_chb):
                for c in range(len(coll_chb) - 1):
                    b0, b1 = coll_chb[c], coll_chb[c + 1]
                    nc.gpsimd.collective_compute(
                        "AllGather", mybir.AluOpType.bypass,
                        replica_groups=[list(range(CORES))],
                        ins=[agin[b0 * P:b1 * P, :]],
                        outs=[tabs[c][:]])

            def emit_writeback(hT, nblk, agin, tabs, coll_chb, dma_chb):
                for b in range(nblk):
                    psr = ps_misc.tile([P, H], BF16, name="psr", tag="misc")
                    nc.tensor.transpose(out=psr[:], in_=hT[:, b * P:(b + 1) * P],
                                        identity=ident_sb[:])
                    nc.vector.tensor_copy(out=rstage[:, b * H:(b + 1) * H],
                                          in_=psr[:])
                for c in range(len(dma_chb) - 1):
                    emit_chunk_dma(agin, dma_chb[c], dma_chb[c + 1])
                emit_chunk_colls(agin, tabs, coll_chb)

            emit_writeback(hvT, NB_V, agin_v, tabs_v[0], VCH, VDM)
            emit_init(cfT, CF, C_S, wcon_sb, bcon_sb, hcT)

            # ---- one message-passing phase
            def emit_phase(tab_srcs, nwin, idx_sb, pdst_sb,
                           calls, units, TBg, blk_groups, nblk, hT, W_sb, b_sb,
                           wb):
                # even/odd row views of the per-window table tensors
                win_ap = {}
                for w in range(nwin):
                    tab2 = tab_srcs[w][:].rearrange("(n two) f -> n (two f)",
                                                    two=2)
                    for h in range(2):
                        win_ap[(w, h)] = tab2[:, h * H:(h + 1) * H]

                accs = {}
                done_groups = [0] * nblk
                if wb is not None:
                    agin, tabs, coll_chb, chb = wb
                    chunk_left = [chb[c + 1] - chb[c]
                                  for c in range(len(chb) - 1)]

                def emit_update(b):
                    ps2 = ps_upd.tile([H, P], F32, name="ps2", tag="ps2")
                    nc.tensor.matmul(out=ps2[:], lhsT=W_sb[:],
                                     rhs=macc[:, b * P:(b + 1) * P],
                                     start=True, stop=True)
                    tmp = mpool.tile([H, P], F32, name="tmp", tag="tmp")
                    nc.vector.tensor_tensor(out=tmp[:], in0=ps2[:],
                                            in1=hT[:, b * P:(b + 1) * P],
                                            op=AluOp.add)
                    nc.scalar.activation(out=hT[:, b * P:(b + 1) * P],
                                         in_=tmp[:], func=Act.Relu, bias=b_sb[:])
                    if wb is not None:
                        psr = ps_misc.tile([P, H], BF16, name="psr", tag="misc")
                        nc.tensor.transpose(out=psr[:],
                                            in_=hT[:, b * P:(b + 1) * P],
                                            identity=ident_sb[:])
                        nc.scalar.activation(
                            out=rstage[:, b * H:(b + 1) * H], in_=psr[:],
                            func=Act.Identity)
                        c = int(np.searchsorted(chb, b, "right")) - 1
                        chunk_left[c] -= 1
                        if chunk_left[c] == 0:
                            emit_chunk_dma(agin, chb[c], chb[c + 1])

                prev_g = None
                for ci, (w, h, t0, nt, ulist) in enumerate(calls):
                    nmu = len(ulist)
                    g = gpool.tile([P, TPC, H], BF16, name="g", tag="g")
                    _dma_gather_raw(
                        nc.gpsimd, g[:, :nt, :], win_ap[(w, h)],
                        idx_sb[:, t0 * 8:(t0 + nt) * 8],
                        num_idxs=nt * P, elem_size=H, elem_step=2 * H,
                        queue_num=ci % 4)
                    if nmu == 0:
                        prev_g = g
                        continue
                    mu0 = ulist[0]
                    S = s_pool.tile([P, TPC, P], BF16, name="S", tag="S")
                    nc.vector.tensor_tensor(
                        out=S[:, :nmu, :],
                        in0=iota_sb[:, None, :].to_broadcast([P, nmu, P]),
                        in1=pdst_sb[:, mu0:mu0 + nmu, None]
                            .to_broadcast([P, nmu, P]),
                        op=AluOp.is_equal)
                    for mi, u in enumerate(ulist):
                        grp, is_pack, uoff, first, last = units[u]
                        lo = int(TBg[grp]) + uoff - t0
                        if is_pack:
                            ss = mpool.tile([P, H], BF16, name="ss", tag="ss",
                                            bufs=4)
                            a_ap = (prev_g[:, TPC + lo, :] if lo < 0
                                    else g[:, lo, :])
                            nc.vector.tensor_tensor(
                                out=ss[:], in0=a_ap, in1=g[:, lo + 1, :],
                                op=AluOp.add)
                            lhs = ss[:]
                        else:
                            lhs = g[:, lo, :]
                        if first:
                            accs[grp] = ps_acc.tile([H, P], F32, name="acc",
                                                    tag="acc")
                        nc.tensor.matmul(out=accs[grp][:], lhsT=lhs,
                                         rhs=S[:, mi, :], start=first, stop=last)
                        if last:
                            b = grp % nblk
                            glist = blk_groups[b]
                            if done_groups[b] == 0:
                                nc.scalar.activation(
                                    out=macc[:, b * P:(b + 1) * P],
                                    in_=accs[grp][:], func=Act.Identity)
                            else:
                                nc.vector.tensor_tensor(
                                    out=macc[:, b * P:(b + 1) * P],
                                    in0=macc[:, b * P:(b + 1) * P],
                                    in1=accs[grp][:], op=AluOp.add)
                            del accs[grp]
                            done_groups[b] += 1
                            if done_groups[b] == len(glist):
                                emit_update(b)
                    prev_g = g
                if wb is not None:
                    emit_chunk_colls(agin, tabs, coll_chb)

            for r in range(ROUNDS):
                emit_phase(tabs_v[r], VWIN, idx_c_sb, pdst_c_sb,
                           calls_c, units_c, TB_c, bg_c, NB_C, hcT, wv2c_sb,
                           bv2c_sb, (agin_c, tabs_c[r], CCH, CDM))
                last = r == ROUNDS - 1
                emit_phase(tabs_c[r], CWIN, idx_v_sb, pdst_v_sb,
                           calls_v, units_v, TB_v, bg_v, NB_V, hvT, wc2v_sb,
                           bc2v_sb, None if last else
                           (agin_v, tabs_v[r + 1], VCH, VDM))

            # ---- scores = h_var @ w_score + b_score (shard)
            c0 = 0
            while c0 < V_S:
                wd = min(512, V_S - c0)
                pss = ps_misc.tile([1, 512], F32, name="pss", tag="misc")
                nc.tensor.matmul(out=pss[:, :wd], lhsT=wsco_sb[:],
                                 rhs=hvT[:, c0:c0 + wd], start=True, stop=True)
                sch = mpool.tile([1, 512], F32, name="sch", tag="sch")
                nc.vector.tensor_scalar(
                    out=sch[:, :wd], in0=pss[:, :wd],
                    scalar1=float(b_score_val), scalar2=None, op0=AluOp.add)
                nc.sync.dma_start(out=scores_out[None, c0:c0 + wd],
                                  in_=sch[0:1, :wd])
                c0 += wd

    nc.compile()
    return nc


_CACHE = {}


def kernel(**inputs):
    var_feat = np.asarray(inputs["var_feat"], np.float32)
    constr_feat = np.asarray(inputs["constr_feat"], np.float32)
    var_idx = np.asarray(inputs["var_idx"]).astype(np.int64)
    constr_idx = np.asarray(inputs["constr_idx"]).astype(np.int64)
    b_score_val = float(np.asarray(inputs["b_score"]).reshape(-1)[0])

    key = (var_idx.tobytes(), constr_idx.tobytes())
    if key in _CACHE:
        nc, idx_v, pdst_v, idx_c, pdst_c = _CACHE[key]
    else:
        rm_v = _row_map(V_CORE, VCH)
        rm_c = _row_map(C_CORE, CCH)
        # v2c: dst=constr, src=var
        idx_v, pdst_v, npk2_c, nraw_c = _prep_direction(
            constr_idx, rm_v[var_idx], C_CORE, NB_C, VWIN, RV // VWIN)
        # c2v: dst=var, src=constr
        idx_c, pdst_c, npk2_v, nraw_v = _prep_direction(
            var_idx, rm_c[constr_idx], V_CORE, NB_V, CWIN, RC // CWIN)
        nc = _build((npk2_c, nraw_c), (npk2_v, nraw_v), b_score_val)
        _CACHE[key] = (nc, idx_v, pdst_v, idx_c, pdst_c)

    iota = np.broadcast_to(np.arange(P, dtype=np.float32),
                           (P, P)).astype(BF).copy()
    ident = np.eye(H, dtype=np.float32).astype(BF)

    vf_pad = np.zeros((CORES, V_S, VF), np.float32)
    vf_pad[:, :V_CORE] = var_feat.reshape(CORES, V_CORE, VF)
    cf_pad = np.zeros((CORES, C_S, CF), np.float32)
    cf_pad[:, :C_CORE] = constr_feat.reshape(CORES, C_CORE, CF)

    common = dict(
        wvar=np.ascontiguousarray(inputs["W_var"], dtype=np.float32),
        wcon=np.ascontiguousarray(inputs["W_con"], dtype=np.float32),
        wv2c=np.ascontiguousarray(inputs["W_v2c"], dtype=np.float32),
        wc2v=np.ascontiguousarray(inputs["W_c2v"], dtype=np.float32),
        wsco=np.ascontiguousarray(inputs["W_score"], dtype=np.float32).astype(BF),
        bvar=np.ascontiguousarray(inputs["b_var"], dtype=np.float32).reshape(H, 1),
        bcon=np.ascontiguousarray(inputs["b_con"], dtype=np.float32).reshape(H, 1),
        bv2c=np.ascontiguousarray(inputs["b_v2c"], dtype=np.float32).reshape(H, 1),
        bc2v=np.ascontiguousarray(inputs["b_c2v"], dtype=np.float32).reshape(H, 1),
        iota=iota, ident=ident,
    )
    in_maps = []
    for k in range(CORES):
        m = dict(common)
        m["vfT"] = np.ascontiguousarray(vf_pad[k].T)
        m["cfT"] = np.ascontiguousarray(cf_pad[k].T)
        m["idx_v2c"] = idx_v[k]
        m["pdst_v2c"] = pdst_v[k]
        m["idx_c2v"] = idx_c[k]
        m["pdst_c2v"] = pdst_c[k]
        in_maps.append(m)

    res = run_bass_kernel_spmd(nc, in_maps, list(range(CORES)))
    scores = np.concatenate([res.results[k]["scores"].reshape(-1)[:V_CORE]
                             for k in range(CORES)])
    return scores.astype(np.float32)
